# revision 9
# baseline (speedup 1.0000x reference)
"""Causal self-attention (B=4, T=2048, C=768, H=12) on 8 TRN2 NeuronCores.

Sharding: DP=4 over batch x TP=2 over heads (6 heads per core).

v2 pipeline (vs v1 baseline at 348us):
  - X^T built with PE transposes (96x 128x128) instead of sync-queue DMA
    transposes (which serialized 117us on the Sync engine and starved PE).
  - All fp32->bf16 cast DMAs batched (one SWDGE issue per tensor / n-block)
    and ordered x-block-0 first so compute starts ~5us in, not 75us.
  - Attention processed in 512-wide q windows per head pair: scores for the
    two heads of a pair land in one [128,2,512] PSUM tile (quadrant-packed
    matmuls), one Exp ACTIVATE covers both heads, PV accumulates per-window
    [65,512] PSUM tiles (V carries a ones column so row 64 = softmax denom).
  - Causal diag masking via bf16 0/1 triangular multiply on P (DVE) instead
    of fp32 -30000 adds on PSUM; mask/identity built on-device.
  - Score PSUM double-buffered + 2 spare banks so the tile scheduler can
    interleave next-pair kqv projection and output projection matmuls into
    the exp-wait gaps: PE never idles long enough for HAM to re-throttle.
  - Emission is software-pipelined: kq(pair p+1) emitted inside attention of
    pair p; output projection of q-window j emitted inside pair-2 window j.

Matmul inputs bf16; accumulation/softmax fp32. Host sums TP partials + bias.
"""

import sys

sys.path.insert(0, "/opt/trn_rl_repo")

from contextlib import ExitStack

import numpy as np

import concourse.bass as bass
import concourse.tile as tile
from concourse import bacc
from concourse import mybir
from concourse.bass import ts
from concourse.bass_utils import run_bass_kernel_spmd
from concourse.masks import make_identity

F32 = mybir.dt.float32
BF16 = mybir.dt.bfloat16

B, T, C = 4, 2048, 768
H, D = 12, 64
HL = 6           # heads per core
FL = HL * D      # 384 local feature dim
NCT = C // 128   # 6 contraction tiles
NT = T // 128    # 16 token tiles
NB = T // 512    # 4 n-blocks (512 tokens each)
NPAIR = HL // 2  # 3 head pairs
NW = 4           # 512-wide q windows


def build_nc():
    nc = bacc.Bacc()
    x_d = nc.declare_dram_parameter("x", [T, C], F32, isOutput=False)
    wk_d = nc.declare_dram_parameter("wk", [C, FL], F32, isOutput=False)
    wq_d = nc.declare_dram_parameter("wq", [C, FL], F32, isOutput=False)
    wv_d = nc.declare_dram_parameter("wv", [C, FL], F32, isOutput=False)
    wp_d = nc.declare_dram_parameter("wp", [FL, C], F32, isOutput=False)
    bk_d = nc.declare_dram_parameter("bk", [FL], F32, isOutput=False)
    bq_d = nc.declare_dram_parameter("bq", [FL], F32, isOutput=False)
    bv_d = nc.declare_dram_parameter("bv", [FL], F32, isOutput=False)
    y_d = nc.declare_dram_parameter("y", [T, C], F32, isOutput=True)

    with tile.TileContext(nc) as tc, ExitStack() as ctx:
        const = ctx.enter_context(tc.tile_pool(name="const", bufs=1))
        wpool = ctx.enter_context(tc.tile_pool(name="wpool", bufs=1))
        big = ctx.enter_context(tc.tile_pool(name="big", bufs=1))
        xin = ctx.enter_context(tc.tile_pool(name="xin", bufs=2))
        xtp = ctx.enter_context(tc.tile_pool(name="xtp", bufs=1))
        ppool = ctx.enter_context(tc.tile_pool(name="ppool", bufs=4))
        small = ctx.enter_context(tc.tile_pool(name="small", bufs=4))
        ypool = ctx.enter_context(tc.tile_pool(name="ypool", bufs=3))
        mmps = ctx.enter_context(tc.tile_pool(name="mmps", bufs=2, space="PSUM"))

        # ---- constants (all built on-device; no mask DMA) ----
        ident = const.tile([128, 128], BF16)
        make_identity(nc, ident)
        # m01[k, q] = 1 where k <= q (causal-valid) else 0; bf16 multiply mask
        m01 = const.tile([128, 128], BF16)
        nc.gpsimd.memset(m01, 1.0)
        nc.gpsimd.affine_select(
            out=m01, in_=m01,
            pattern=[[1, 128]], channel_multiplier=-1, base=0,
            compare_op=mybir.AluOpType.is_ge, fill=0.0,
        )
        ones_sb = const.tile([1, 128], BF16)
        nc.vector.memset(ones_sb, 1.0)

        # ---- input DMAs: x block 0 first, then weights interleaved ----
        xr = x_d.rearrange("(n tt p) c -> n p tt c", p=128, tt=4)
        xb = [
            xin.tile([128, 4, C], BF16, tag="xb", name=f"xb{n}") for n in range(NB)
        ]
        wkt = wpool.tile([128, NCT, FL], BF16, name="wkt")
        wqt = wpool.tile([128, NCT, FL], BF16, name="wqt")
        wvt = wpool.tile([128, NCT, FL], BF16, name="wvt")
        wpt = wpool.tile([128, NPAIR, C], BF16, name="wpt")
        bk_sb = const.tile([128, NPAIR], F32)
        bq_sb = const.tile([128, NPAIR], F32)
        bv_sb = const.tile([1, FL], BF16)

        nc.gpsimd.dma_start(out=xb[0], in_=xr[0])
        nc.gpsimd.dma_start(out=wkt, in_=wk_d.rearrange("(ct p) f -> p ct f", p=128))
        nc.gpsimd.dma_start(out=wqt, in_=wq_d.rearrange("(ct p) f -> p ct f", p=128))
        nc.gpsimd.dma_start(out=xb[1], in_=xr[1])
        nc.gpsimd.dma_start(out=wvt, in_=wv_d.rearrange("(ct p) f -> p ct f", p=128))
        nc.gpsimd.dma_start(out=bv_sb, in_=bv_d.rearrange("(o f) -> o f", o=1))
        nc.gpsimd.dma_start(out=xb[2], in_=xr[2])
        nc.gpsimd.dma_start(out=wpt, in_=wp_d.rearrange("(i p) c -> p i c", p=128))
        nc.gpsimd.dma_start(out=xb[3], in_=xr[3])
        nc.sync.dma_start(out=bk_sb, in_=bk_d.rearrange("(i p) -> p i", p=128))
        nc.sync.dma_start(out=bq_sb, in_=bq_d.rearrange("(i p) -> p i", p=128))

        # ---- persistent activations ----
        kt_sb = [
            big.tile([128, T], BF16, tag="ktq", bufs=2 * NPAIR, name=f"ktp{i}")
            for i in range(NPAIR)
        ]
        qt_sb = [
            big.tile([128, T], BF16, tag="ktq", bufs=2 * NPAIR, name=f"qtp{i}")
            for i in range(NPAIR)
        ]
        v_sb = [
            big.tile([128, HL, D + 1], BF16, tag="v", bufs=NT, name=f"v{t}")
            for t in range(NT)
        ]
        otn_sb = [
            big.tile([128, T], BF16, tag="otn", bufs=NPAIR, name=f"otn{i}")
            for i in range(NPAIR)
        ]
        xt_sb = [
            [
                xtp.tile([128, 512], BF16, tag="xt", bufs=NB * NCT, name=f"xt{n}_{ct}")
                for ct in range(NCT)
            ]
            for n in range(NB)
        ]

        # ---- phase A: X^T via PE transposes (scoped PSUM pool) ----
        with tc.tile_pool(name="tpps", bufs=2, space="PSUM") as tpps:
            for n in range(NB):
                for ct in range(NCT):
                    tp = tpps.tile([128, 512], BF16, tag="tp", name=f"tp{n}_{ct}")
                    for tt in range(4):
                        nc.tensor.transpose(
                            out=tp[:, ts(tt, 128)],
                            in_=xb[n][:, tt, ts(ct, 128)],
                            identity=ident,
                        )
                    nc.vector.tensor_copy(out=xt_sb[n][ct], in_=tp)

            # ---- phase B: V projection (natural layout + ones column) ----
            for t in range(NT):
                n, tt = t // 4, t % 4
                psv = mmps.tile([128, 512], F32, tag="mm", name=f"vps{t}")
                for ct in range(NCT):
                    nc.tensor.matmul(
                        out=psv[:, 0:FL],
                        lhsT=xt_sb[n][ct][:, ts(tt, 128)],
                        rhs=wvt[:, ct, :],
                        start=(ct == 0),
                        stop=False,
                    )
                nc.tensor.matmul(
                    out=psv[:, 0:FL],
                    lhsT=ones_sb,
                    rhs=bv_sb,
                    start=False,
                    stop=True,
                )
                nc.vector.tensor_copy(
                    out=v_sb[t][:, :, 0:D],
                    in_=psv[:, 0:FL].rearrange("p (h d) -> p h d", h=HL),
                )
                nc.gpsimd.memset(v_sb[t][:, :, D : D + 1], 1.0)

            # ---- phase C: K^T/Q^T projection for pair 0 ----
            def kq_group(pair, n, which):
                """One [128,512] block of K^T (which=0) or Q^T (which=1)."""
                w_src, b_src, dest = (
                    (wkt, bk_sb, kt_sb) if which == 0 else (wqt, bq_sb, qt_sb)
                )
                ps = mmps.tile([128, 512], F32, tag="mm", name=f"kq{pair}_{n}_{which}")
                for ct in range(NCT):
                    nc.tensor.matmul(
                        out=ps,
                        lhsT=w_src[:, ct, ts(pair, 128)],
                        rhs=xt_sb[n][ct],
                        start=(ct == 0),
                        stop=(ct == NCT - 1),
                    )
                nc.vector.tensor_scalar_add(
                    out=dest[pair][:, ts(n, 512)],
                    in0=ps,
                    scalar1=b_src[:, pair : pair + 1],
                )

            for p in range(NPAIR):
                kq_group(p, 0, 0)
                kq_group(p, 0, 1)

        # ---- attention PSUM pools (alloc after transpose pool release) ----
        spool = ctx.enter_context(tc.tile_pool(name="spool", bufs=2, space="PSUM"))
        otps_pool = ctx.enter_context(tc.tile_pool(name="otps", bufs=2, space="PSUM"))

        # remaining kq groups: chunk n is first needed by window round j=n,
        # so emit the six (pair, n=j+1) groups as PE filler inside round j.
        kq_rest = [(p, n, w) for n in (1, 2, 3) for p in range(NPAIR) for w in (0, 1)]
        kq_i = 0

        def proj_window(j):
            """Output projection + store for q tiles of window j (needs all pairs)."""
            for qi in range(4 * j, 4 * j + 4):
                y_sb = ypool.tile([128, C], F32, tag="y", name=f"y{qi}")
                for half in range(2):
                    fps = mmps.tile([128, 512], F32, tag="mm", name=f"fp{qi}_{half}")
                    for pair in range(NPAIR):
                        nc.tensor.matmul(
                            out=fps[:, 0:FL],
                            lhsT=otn_sb[pair][:, ts(qi, 128)],
                            rhs=wpt[:, pair, ts(half, FL)],
                            start=(pair == 0),
                            stop=(pair == NPAIR - 1),
                        )
                    nc.vector.tensor_copy(out=y_sb[:, ts(half, FL)], in_=fps[:, 0:FL])
                nc.sync.dma_start(out=y_d[ts(qi, 128), :], in_=y_sb)

        for j in range(NW):
            for pair in range(NPAIR):
                q0 = 512 * j
                # keep PE fed during exp waits: next round's kq projections
                for _ in range(2):
                    if kq_i < len(kq_rest):
                        p, n, w = kq_rest[kq_i]
                        if n == j + 1:
                            kq_group(p, n, w)
                            kq_i += 1
                nkt = 4 * j + 4
                ot_h = {}
                for h in (0, 1):
                    ot_h[h] = otps_pool.tile(
                        [D + 1, 512], F32, tag="ot", name=f"ot{pair}_{j}_{h}"
                    )
                for kt in range(nkt):
                    c0 = 128 * kt
                    s0 = max(q0, c0) - q0       # ragged start within window
                    w = 512 - s0
                    diag = kt >= 4 * j          # this k tile hits the diagonal
                    sps = spool.tile([128, 2, 512], F32, tag="s", name=f"s{pair}_{j}_{kt}")
                    for h in (0, 1):
                        row0 = 64 * h
                        nc.tensor.matmul(
                            out=sps[:, h, s0:512],
                            lhsT=kt_sb[pair][row0 : row0 + 64, ts(kt, 128)],
                            rhs=qt_sb[pair][row0 : row0 + 64, q0 + s0 : q0 + 512],
                            start=True,
                            stop=True,
                            tile_position=(row0, 0),
                            skip_group_check=True,
                        )
                    pb = ppool.tile([128, 2, 512], BF16, tag="p", name=f"p{pair}_{j}_{kt}")
                    nc.scalar.activation(
                        out=pb[:, :, s0:512],
                        in_=sps[:, :, s0:512],
                        func=mybir.ActivationFunctionType.Exp,
                        scale=float(D) ** -0.5,
                    )
                    if diag:
                        # zero strict-lower (k > q) of the 128-wide diag block
                        for h in (0, 1):
                            nc.gpsimd.tensor_mul(
                                pb[:, h, s0 : s0 + 128],
                                pb[:, h, s0 : s0 + 128],
                                m01,
                            )
                    for h in (0, 1):
                        nc.tensor.matmul(
                            out=ot_h[h][:, s0:512],
                            lhsT=v_sb[kt][:, 2 * pair + h, :],
                            rhs=pb[:, h, s0:512],
                            start=(kt == 0),
                            stop=(kt == nkt - 1),
                        )
                # normalize: row D of ot is the softmax denominator l
                for h in (0, 1):
                    row0 = 64 * h
                    lv = small.tile([1, 512], F32, tag="lv", name=f"lv{pair}_{j}_{h}")
                    nc.vector.tensor_copy(out=lv, in_=ot_h[h][D : D + 1, :])
                    rv = small.tile([1, 512], F32, tag="l", name=f"l{pair}_{j}_{h}")
                    nc.vector.reciprocal_approx_fast(out=rv, in_=lv)
                    rb = small.tile([64, 512], F32, tag="rb", name=f"rb{pair}_{j}_{h}")
                    nc.gpsimd.partition_broadcast(rb, rv)
                    nc.vector.tensor_mul(
                        otn_sb[pair][row0 : row0 + 64, ts(j, 512)],
                        ot_h[h][0:D, :],
                        rb,
                    )
                if pair == NPAIR - 1:
                    proj_window(j)

    nc.compile()
    return nc


_NC = None


def _get_nc():
    global _NC
    if _NC is None:
        _NC = build_nc()
    return _NC


def make_in_maps(x, W_kqv, b_kqv, W_proj):
    in_maps = []
    for core in range(8):
        b = core // 2
        h0 = (core % 2) * HL * D  # feature offset of this core's head group
        in_maps.append(
            {
                "x": np.ascontiguousarray(x[b]),
                "wk": np.ascontiguousarray(W_kqv[:, h0 : h0 + FL]),
                "wq": np.ascontiguousarray(W_kqv[:, C + h0 : C + h0 + FL]),
                "wv": np.ascontiguousarray(W_kqv[:, 2 * C + h0 : 2 * C + h0 + FL]),
                "wp": np.ascontiguousarray(W_proj[h0 : h0 + FL, :]),
                "bk": np.ascontiguousarray(b_kqv[h0 : h0 + FL]),
                "bq": np.ascontiguousarray(b_kqv[C + h0 : C + h0 + FL]),
                "bv": np.ascontiguousarray(b_kqv[2 * C + h0 : 2 * C + h0 + FL]),
            }
        )
    return in_maps


def _combine(results, b_proj):
    y = np.empty((B, T, C), dtype=np.float32)
    for b in range(B):
        y[b] = results[2 * b]["y"] + results[2 * b + 1]["y"] + b_proj[None, :]
    return y


def kernel(x, W_kqv, b_kqv, W_proj, b_proj, **run_kwargs):
    x = np.asarray(x, dtype=np.float32)
    W_kqv = np.asarray(W_kqv, dtype=np.float32)
    b_kqv = np.asarray(b_kqv, dtype=np.float32)
    W_proj = np.asarray(W_proj, dtype=np.float32)
    b_proj = np.asarray(b_proj, dtype=np.float32)

    nc = _get_nc()
    in_maps = make_in_maps(x, W_kqv, b_kqv, W_proj)
    res = run_bass_kernel_spmd(nc, in_maps, core_ids=list(range(8)), **run_kwargs)
    out = _combine(res.results, b_proj)
    kernel.last_result = res
    return out
